# revision 4
# baseline (speedup 1.0000x reference)
"""Fused cross-attention kernel for TRN2, data-parallel over batch (8 cores).

Per core (one batch item):
  A  = relu(XA @ W.T + b)          [2048, 512]
  Bf = relu(XB @ W.T + b)          [2048, 512]
  S  = A @ Bf.T                    [2048, 2048]
  Sm = where(maskA[:,None]*maskB[None,:]==0, -1e9, S)
  attnA = softmax(Sm, axis=0); attnB = softmax(Sm, axis=1)
  cvA = attnA.T @ A                [2048, 512]
  cvB = attnB @ Bf                 [2048, 512]

Softmax trick (matches the -1e9 + max-subtract reference semantics exactly,
including the all-masked uniform rows/cols):
  x1[l,m] = (S*ma[l] + (C*ma[l] - Gs)) * mb[m]      EA = exp(x1)
  x2[m,l] = (S*mb[m] + (C*mb[m] - Gs)) * ma[l]      EB = exp(x2)
with C=1000 and Gs = max over tile-0 of (S+C)*ma (a global-ish stabilizer:
softmax is shift-invariant per column/row; masked entries get exponent
~-1e3 -> exp 0; fully-masked columns/rows get exponent exactly 0 -> exp 1,
reproducing the uniform distribution the reference produces there).
  cvA[m,:] = (EA.T @ A) / sum_l EA ;  cvB[l,:] = (EB @ Bf) / sum_m EB
"""

import os
import sys
import numpy as np
from contextlib import ExitStack

if "/opt/trn_rl_repo" not in sys.path:
    sys.path.insert(0, "/opt/trn_rl_repo")

PHASE_LIMIT = int(os.environ.get("PHASE_LIMIT", "7"))

P = 128
L = 2048          # LA == LB
D = 512           # input feature dim
H = 512           # hidden dim
NLT = L // P      # 16 l-tiles (and m-tiles)
NHT = H // P      # 4 h-chunks
NDT = D // P      # 4 d-chunks
MASK_C = 1000.0   # small shift constant (fp32-safe, >> exp underflow span)
N_CORES = 8


def build_kernel_body(ctx, tc, outs, ins):
    """Emit the per-core program. outs/ins are dicts name -> bass.AP (DRAM)."""
    import concourse.bass as bass
    import concourse.mybir as mybir
    from concourse import bass_isa
    from concourse.masks import make_identity

    nc = tc.nc
    f32 = mybir.dt.float32
    f16 = mybir.dt.float16
    bf16 = mybir.dt.bfloat16
    AX = mybir.AxisListType
    OP = mybir.AluOpType
    ACT_T = mybir.ActivationFunctionType

    xa, xb = ins["xa"], ins["xb"]
    ma_d, mb_d = ins["ma"], ins["mb"]
    w_d, b_d = ins["w"], ins["bias"]
    cva_d, cvb_d = outs["cva"], outs["cvb"]

    # ---------------- persistent pools ----------------
    const = ctx.enter_context(tc.tile_pool(name="const", bufs=1))
    big = ctx.enter_context(tc.tile_pool(name="big", bufs=1))

    ident = const.tile([P, P], f16)
    make_identity(nc, ident)
    ones_bf = const.tile([P, 1], bf16)
    nc.gpsimd.memset(ones_bf[:], 1.0)

    ma_cols = const.tile([P, NLT], f32)   # ma per-partition, one col per l-tile
    mb_cols = const.tile([P, NLT], f32)
    maC = const.tile([P, NLT], f32)       # C * ma
    mbC = const.tile([P, NLT], f32)
    maoff = const.tile([P, NLT], f32)     # C*ma - Gshift
    mboff = const.tile([P, NLT], f32)
    gsh = const.tile([P, 1], f32)         # Gshift broadcast to all partitions
    b_cols = const.tile([P, NHT], f32)
    ma_row = const.tile([1, L], f32)
    mb_row = const.tile([1, L], f32)
    ma_bc = const.tile([P, L], f32)       # maskA broadcast along partitions
    mb_bc = const.tile([P, L], f32)

    for i in range(NLT):
        nc.sync.dma_start(ma_cols[:, i : i + 1], ma_d[P * i : P * (i + 1), 0:1])
        nc.sync.dma_start(mb_cols[:, i : i + 1], mb_d[P * i : P * (i + 1), 0:1])
    for j in range(NHT):
        nc.sync.dma_start(b_cols[:, j : j + 1], b_d[P * j : P * (j + 1), 0:1])
    nc.sync.dma_start(ma_row[:, :], ma_d.rearrange("a b -> b a"))
    nc.sync.dma_start(mb_row[:, :], mb_d.rearrange("a b -> b a"))
    nc.gpsimd.partition_broadcast(ma_bc[:], ma_row[:], channels=P)
    nc.gpsimd.partition_broadcast(mb_bc[:], mb_row[:], channels=P)
    nc.vector.tensor_scalar_mul(maC[:], ma_cols[:], MASK_C)
    nc.vector.tensor_scalar_mul(mbC[:], mb_cols[:], MASK_C)

    # A^T / Bf^T (post-relu projections, [h, l] orientation), fp16
    AT = big.tile([P, NHT * L], f16)   # chunk hk -> AT[:, hk*L:(hk+1)*L]
    BT = big.tile([P, NHT * L], f16)
    # natural orientation copies for the cv matmuls, bf16
    Alh = big.tile([P, NLT * H], bf16)  # l-tile li -> Alh[:, li*H:(li+1)*H]
    Bmh = big.tile([P, NLT * H], bf16)

    # ---------------- phase 0-3: W, input transposes, projections ----------
    with (
        tc.tile_pool(name="wt", bufs=1) as wtp,
        tc.tile_pool(name="xat", bufs=1) as xatp,
        tc.tile_pool(name="ld", bufs=3) as ldp,
        tc.tile_pool(name="c16", bufs=3) as c16p,
        tc.tile_pool(name="pt", bufs=4, space="PSUM") as ptp,
        tc.tile_pool(name="pbig0", bufs=1, space="PSUM") as pbig0,
    ):
        WT = wtp.tile([P, NDT * H], f16)   # W^T: d-chunk dk -> WT[:, dk*H:(dk+1)*H]
        for hj in range(NHT):
            wld = ldp.tile([P, D], f32, tag="ld")
            nc.sync.dma_start(wld[:], w_d[P * hj : P * (hj + 1), :])
            w16 = c16p.tile([P, D], f16, tag="c16")
            nc.vector.tensor_copy(w16[:], wld[:])
            for dk in range(NDT):
                pt = ptp.tile([P, P], f16, tag="pt")
                nc.tensor.transpose(pt[:], w16[:, P * dk : P * (dk + 1)], ident[:])
                nc.scalar.copy(WT[:, dk * H + P * hj : dk * H + P * (hj + 1)], pt[:])

        XAT = xatp.tile([P, NDT * L], f16)  # XA^T
        XBT = xatp.tile([P, NDT * L], f16)
        for src, dst in ((xa, XAT), (xb, XBT)):
            for li in range(NLT):
                ld = ldp.tile([P, D], f32, tag="ld")
                nc.sync.dma_start(ld[:], src[P * li : P * (li + 1), :])
                c16 = c16p.tile([P, D], f16, tag="c16")
                nc.vector.tensor_copy(c16[:], ld[:])
                for dk in range(NDT):
                    pt = ptp.tile([P, P], f16, tag="pt")
                    nc.tensor.transpose(pt[:], c16[:, P * dk : P * (dk + 1)], ident[:])
                    nc.scalar.copy(
                        dst[:, dk * L + P * li : dk * L + P * (li + 1)], pt[:]
                    )

        # projections: out A^T[h-chunk hj] = relu(W^T.T @ XA^T + b)
        for XT, OT in ((XAT, AT), (XBT, BT)):
            for hj in range(NHT):
                ps = pbig0.tile([P, L], f32, tag="pj")
                for lj in range(L // 512):
                    for dk in range(NDT):
                        nc.tensor.matmul(
                            ps[:, 512 * lj : 512 * (lj + 1)],
                            WT[:, dk * H + P * hj : dk * H + P * (hj + 1)],
                            XT[:, dk * L + 512 * lj : dk * L + 512 * (lj + 1)],
                            start=(dk == 0),
                            stop=(dk == NDT - 1),
                        )
                nc.scalar.activation(
                    OT[:, hj * L : (hj + 1) * L],
                    ps[:],
                    ACT_T.Relu,
                    bias=b_cols[:, hj : hj + 1],
                )

        # natural-orientation bf16 copies via PE transpose of A^T / Bf^T
        for SRC, DST in ((AT, Alh), (BT, Bmh)):
            for li in range(NLT):
                for hk in range(NHT):
                    pt = ptp.tile([P, P], f16, tag="pt")
                    nc.tensor.transpose(
                        pt[:], SRC[:, hk * L + P * li : hk * L + P * (li + 1)], ident[:]
                    )
                    nc.scalar.copy(
                        DST[:, li * H + P * hk : li * H + P * (hk + 1)], pt[:]
                    )

    if PHASE_LIMIT < 4:
        return

    # E buffer: holds EA (bf16) for phase 4-5, then reused for EB in 6-7.
    E = big.tile([P, NLT * L], bf16)

    # ---------------- phase 4: S1 -> EA ----------------
    with (
        tc.tile_pool(name="pbig", bufs=2, space="PSUM") as pbig,
        tc.tile_pool(name="xw", bufs=3) as xwp,
        tc.tile_pool(name="st", bufs=2) as stp,
    ):
        for li in range(NLT):
            ps = pbig.tile([P, L], f32, tag="s")
            for mj in range(L // 512):
                for hk in range(NHT):
                    nc.tensor.matmul(
                        ps[:, 512 * mj : 512 * (mj + 1)],
                        AT[:, hk * L + P * li : hk * L + P * (li + 1)],
                        BT[:, hk * L + 512 * mj : hk * L + 512 * (mj + 1)],
                        start=(hk == 0),
                        stop=(hk == NHT - 1),
                    )
            if li == 0:
                # Gshift = max over tile-0 of (S + C) * ma   (in all partitions)
                t0 = xwp.tile([P, L], f32, tag="xw")
                nc.vector.tensor_scalar(
                    t0[:], ps[:], ma_cols[:, 0:1], maC[:, 0:1], OP.mult, OP.add
                )
                rm = stp.tile([P, 1], f32, tag="st")
                nc.vector.tensor_reduce(rm[:], t0[:], axis=AX.X, op=OP.max)
                nc.gpsimd.partition_all_reduce(
                    gsh[:], rm[:], channels=P, reduce_op=bass_isa.ReduceOp.max
                )
                nc.vector.tensor_scalar(
                    maoff[:], maC[:], gsh[:], None, OP.subtract
                )
                nc.vector.tensor_scalar(
                    mboff[:], mbC[:], gsh[:], None, OP.subtract
                )
            xt = xwp.tile([P, L], f32, tag="xw")
            nc.vector.tensor_scalar(
                xt[:], ps[:], ma_cols[:, li : li + 1], maoff[:, li : li + 1],
                OP.mult, OP.add,
            )
            nc.vector.tensor_tensor(xt[:], xt[:], mb_bc[:], op=OP.mult)
            nc.scalar.activation(E[:, li * L : (li + 1) * L], xt[:], ACT_T.Exp)

    if PHASE_LIMIT < 5:
        return

    # ---------------- phase 5: cvA ----------------
    with (
        tc.tile_pool(name="pcv", bufs=2, space="PSUM") as pcvp,
        tc.tile_pool(name="psm", bufs=2, space="PSUM") as psmp,
        tc.tile_pool(name="co", bufs=3) as cop,
        tc.tile_pool(name="rc", bufs=2) as rcp,
    ):
        for mj in range(NLT):
            pcv = pcvp.tile([P, H], f32, tag="cv")
            psm = psmp.tile([P, 1], f32, tag="sm")
            for li in range(NLT):
                lhsT = E[:, li * L + P * mj : li * L + P * (mj + 1)]
                nc.tensor.matmul(
                    pcv[:], lhsT, Alh[:, li * H : (li + 1) * H],
                    start=(li == 0), stop=(li == NLT - 1),
                )
                nc.tensor.matmul(
                    psm[:], lhsT, ones_bf[:],
                    start=(li == 0), stop=(li == NLT - 1),
                )
            rc = rcp.tile([P, 1], f32, tag="rc")
            nc.vector.reciprocal(rc[:], psm[:])
            co = cop.tile([P, H], f32, tag="co")
            nc.vector.tensor_scalar(co[:], pcv[:], rc[:], None, OP.mult)
            nc.sync.dma_start(cva_d[P * mj : P * (mj + 1), :], co[:])

    if PHASE_LIMIT < 6:
        return

    # ---------------- phase 6: S2 -> EB (reuses E) ----------------
    with (
        tc.tile_pool(name="pbig2", bufs=2, space="PSUM") as pbig2,
        tc.tile_pool(name="xw2", bufs=3) as xwp2,
    ):
        for mj in range(NLT):
            ps = pbig2.tile([P, L], f32, tag="s2")
            for lj in range(L // 512):
                for hk in range(NHT):
                    nc.tensor.matmul(
                        ps[:, 512 * lj : 512 * (lj + 1)],
                        BT[:, hk * L + P * mj : hk * L + P * (mj + 1)],
                        AT[:, hk * L + 512 * lj : hk * L + 512 * (lj + 1)],
                        start=(hk == 0),
                        stop=(hk == NHT - 1),
                    )
            xt = xwp2.tile([P, L], f32, tag="xw2")
            nc.vector.tensor_scalar(
                xt[:], ps[:], mb_cols[:, mj : mj + 1], mboff[:, mj : mj + 1],
                OP.mult, OP.add,
            )
            nc.vector.tensor_tensor(xt[:], xt[:], ma_bc[:], op=OP.mult)
            nc.scalar.activation(E[:, mj * L : (mj + 1) * L], xt[:], ACT_T.Exp)

    if PHASE_LIMIT < 7:
        return

    # ---------------- phase 7: cvB ----------------
    with (
        tc.tile_pool(name="pcv2", bufs=2, space="PSUM") as pcvp2,
        tc.tile_pool(name="psm2", bufs=2, space="PSUM") as psmp2,
        tc.tile_pool(name="co2", bufs=3) as cop2,
        tc.tile_pool(name="rc2", bufs=2) as rcp2,
    ):
        for li in range(NLT):
            pcv = pcvp2.tile([P, H], f32, tag="cv2")
            psm = psmp2.tile([P, 1], f32, tag="sm2")
            for mj in range(NLT):
                lhsT = E[:, mj * L + P * li : mj * L + P * (li + 1)]
                nc.tensor.matmul(
                    pcv[:], lhsT, Bmh[:, mj * H : (mj + 1) * H],
                    start=(mj == 0), stop=(mj == NLT - 1),
                )
                nc.tensor.matmul(
                    psm[:], lhsT, ones_bf[:],
                    start=(mj == 0), stop=(mj == NLT - 1),
                )
            rc = rcp2.tile([P, 1], f32, tag="rc2")
            nc.vector.reciprocal(rc[:], psm[:])
            co = cop2.tile([P, H], f32, tag="co2")
            nc.vector.tensor_scalar(co[:], pcv[:], rc[:], None, OP.mult)
            nc.sync.dma_start(cvb_d[P * li : P * (li + 1), :], co[:])


_CACHED = {}


def _build_program():
    if "nc" in _CACHED:
        return _CACHED["nc"]
    from concourse import bacc
    import concourse.bass as bass
    import concourse.mybir as mybir
    import concourse.tile as tile

    f32 = mybir.dt.float32
    nc = bacc.Bacc(
        "TRN2",
        target_bir_lowering=False,
        debug=False,
        enable_asserts=False,
        num_devices=N_CORES,
    )
    ins = {
        "xa": nc.dram_tensor("xa", [L, D], f32, kind="ExternalInput").ap(),
        "xb": nc.dram_tensor("xb", [L, D], f32, kind="ExternalInput").ap(),
        "ma": nc.dram_tensor("ma", [L, 1], f32, kind="ExternalInput").ap(),
        "mb": nc.dram_tensor("mb", [L, 1], f32, kind="ExternalInput").ap(),
        "w": nc.dram_tensor("w", [H, D], f32, kind="ExternalInput").ap(),
        "bias": nc.dram_tensor("bias", [H, 1], f32, kind="ExternalInput").ap(),
    }
    outs = {
        "cva": nc.dram_tensor("cva", [L, H], f32, kind="ExternalOutput").ap(),
        "cvb": nc.dram_tensor("cvb", [L, H], f32, kind="ExternalOutput").ap(),
    }
    with tile.TileContext(nc) as tc:
        with ExitStack() as ctx:
            build_kernel_body(ctx, tc, outs, ins)
    nc.compile()
    _CACHED["nc"] = nc
    return nc


def run_on_cores(inputA, inputB, maskA, maskB, W, b, trace=False):
    from concourse import bass_utils

    nc = _build_program()
    in_maps = []
    for c in range(N_CORES):
        in_maps.append(
            {
                "xa": np.ascontiguousarray(inputA[c], dtype=np.float32),
                "xb": np.ascontiguousarray(inputB[c], dtype=np.float32),
                "ma": np.ascontiguousarray(
                    maskA[c], dtype=np.float32
                ).reshape(L, 1),
                "mb": np.ascontiguousarray(
                    maskB[c], dtype=np.float32
                ).reshape(L, 1),
                "w": np.ascontiguousarray(W, dtype=np.float32),
                "bias": np.ascontiguousarray(b, dtype=np.float32).reshape(H, 1),
            }
        )
    res = bass_utils.run_bass_kernel_spmd(
        nc, in_maps, core_ids=list(range(N_CORES)), trace=trace
    )
    cvA = np.stack([res.results[c]["cva"] for c in range(N_CORES)])
    cvB = np.stack([res.results[c]["cvb"] for c in range(N_CORES)])
    return (cvA, cvB), res


def kernel(inputA, inputB, maskA, maskB, W, b):
    (cvA, cvB), _ = run_on_cores(inputA, inputB, maskA, maskB, W, b, trace=False)
    return cvA, cvB
